# revision 23
# baseline (speedup 1.0000x reference)
"""v3: all-fp8 (e4m3) DoubleRow matmuls for all 7 attentions; transposed
scores (keys on partitions, no probability transposes); softmax sums via
ones-matmul into PSUM + PE transpose of the [1,L] sums row; interleaved
score/PV phases of the two symmetric attentions to hide exp latency.
Sim-predicted rel err ~7.6e-3 (gate 2e-2)."""

import numpy as np
import ml_dtypes

from concourse import bacc, bass, tile, mybir
from concourse.bass_utils import run_bass_kernel_spmd

B, L, D = 16, 1024, 512
A = D
NCORES = 8
BLOC = B // NCORES
P = 128
DC = D // P
AC = A // P
LT = L // P
KC = L // P
NH = 512
SCALE = float(1.0 / np.sqrt(np.float32(D)))
GS = 8.0  # host-side scale on the fused G = Wk^T @ Wq matrices
SSCALE = SCALE / GS
EXP_BIAS = -1.0   # symmetric attns: keeps exp() under fp8e4 max 240
CROSS_BIAS = -5.0  # cross attn has wider score range (queries = av)

F32 = mybir.dt.float32
BF16 = mybir.dt.bfloat16
F8 = mybir.dt.float8e4
DR = mybir.MatmulPerfMode.DoubleRow
EXP = mybir.ActivationFunctionType.Exp
COPY = mybir.ActivationFunctionType.Copy
MULT = mybir.AluOpType.mult
ADD = mybir.AluOpType.add

W_NAMES = [f"{blk}_{w}" for blk in ("ta", "va", "tv")
           for w in ("kx", "qx", "vx", "ky", "qy", "vy")] + [
    "tav_k", "tav_q", "tav_v"]


def _build():
    nc = bacc.Bacc("TRN2", target_bir_lowering=False, debug=False,
                   num_devices=NCORES)

    mt_txt = nc.dram_tensor("mt_txt", (BLOC, D, L), F8, kind="ExternalInput").ap()
    mt_au = nc.dram_tensor("mt_au", (BLOC, D, L), F8, kind="ExternalInput").ap()
    mt_vi = nc.dram_tensor("mt_vi", (BLOC, D, L), F8, kind="ExternalInput").ap()
    res = nc.dram_tensor("res", (3, BLOC, L, D), F32, kind="ExternalInput").ap()
    wt8 = nc.dram_tensor("wt8", (14, D, A), F8, kind="ExternalInput").ap()
    ident128 = nc.dram_tensor("ident128", (P, P), F32, kind="ExternalInput").ap()
    out = nc.dram_tensor("out", (BLOC, L, 4 * A), F32, kind="ExternalOutput").ap()
    avscr = nc.dram_tensor("avscr", (BLOC, L, A), BF16, kind="ExternalOutput").ap()

    with tile.TileContext(nc) as tc:
        _body(nc, tc, mt_txt, mt_au, mt_vi, res, wt8, ident128, out, avscr)

    nc.compile()
    return nc


def _body(nc, tc, mt_txt, mt_au, mt_vi, res, wt8, ident128, out, avscr):
    mt_dram = {"txt": mt_txt, "au": mt_au, "vi": mt_vi}

    with (
        tc.tile_pool(name="persist", bufs=1) as persist,
        tc.tile_pool(name="wpool", bufs=1) as wpool,
        tc.tile_pool(name="mpool", bufs=1) as mpool,
        tc.tile_pool(name="proj", bufs=1) as projp,
        tc.tile_pool(name="attn", bufs=2) as attnp,
        tc.tile_pool(name="small", bufs=3) as smallp,
        tc.tile_pool(name="ps_a", bufs=2, space=bass.MemorySpace.PSUM) as psA,
        tc.tile_pool(name="ps_b", bufs=2, space=bass.MemorySpace.PSUM) as psB,
    ):
        avT = [persist.tile([P, AC, L], BF16, tag=f"avT{b}", name=f"avT{b}")
               for b in range(BLOC)]
        avT8 = [persist.tile([P, AC, L], F8, tag=f"avT8{b}", name=f"avT8{b}")
                for b in range(BLOC)]
        # pad rows to 32B so the DoubleRow dual-fp8 ldweights stride is legal
        ones8 = persist.tile([P, KC, 32], F8, tag="ones8", name="ones8")
        nc.gpsimd.memset(ones8[:, :, :], 1.0)
        ebias = persist.tile([P, 1], F32, tag="ebias", name="ebias")
        nc.gpsimd.memset(ebias[:, :], EXP_BIAS)
        cbias = persist.tile([P, 1], F32, tag="cbias", name="cbias")
        nc.gpsimd.memset(cbias[:, :], CROSS_BIAS)
        idn = persist.tile([P, P], F32, tag="idn", name="idn")
        nc.sync.dma_start(out=idn[:, :], in_=ident128)

        # all PSUM->SBUF projection copies on DVE: the scalar queue stays
        # shallow so exps and sums copies (which gate PV) retire promptly
        def copy_eng():
            return nc.vector.tensor_copy

        def load_w8(j, slot):
            t = wpool.tile([P, DC, A], F8, tag=f"w{slot}", name=f"w{j}")
            nc.sync.dma_start(out=t[:, :, :],
                              in_=wt8[j].rearrange("(dc p) a -> p dc a", p=P))
            return t

        def load_mt8(name, b, slot, par):
            t = mpool.tile([P, DC, L], F8, tag=f"mT{slot}_{b}_{par}",
                           name=f"mT_{name}{b}")
            src = mt_dram[name][b].rearrange("(dc p) l -> p dc l", p=P)
            nc.sync.dma_start(out=t[:, 0:2, :], in_=src[:, 0:2, :])
            nc.sync.dma_start(out=t[:, 2:4, :], in_=src[:, 2:4, :])
            return t

        def proj_T(wtile, mtile, tag):
            o = projp.tile([P, AC, L], F8, tag=tag, name=tag)
            for ac in range(AC):
                ps = psA.tile([P, 2, NH], F32, tag="psA", name="ps_pt")
                for h in range(2):
                    for dc in (0, 2):
                        nc.tensor.matmul(
                            ps[:, h, :],
                            wtile[:, dc:dc + 2, ac * P:(ac + 1) * P],
                            mtile[:, dc:dc + 2, h * NH:(h + 1) * NH],
                            start=(dc == 0), stop=(dc == 2), perf_mode=DR)
                copy_eng()(
                    o[:, ac, :].rearrange("p (h x) -> p h x", h=2), ps[:, :, :])
            return o

        def proj_N(wtile, mtile, tag):
            o = projp.tile([P, KC, A], F8, tag=tag, name=tag)
            for lt2 in range(0, LT, 2):
                ps = psA.tile([P, 2, A], F32, tag="psA", name="ps_pn")
                for j in range(2):
                    lt = lt2 + j
                    for dc in (0, 2):
                        nc.tensor.matmul(
                            ps[:, j, :],
                            mtile[:, dc:dc + 2, lt * P:(lt + 1) * P],
                            wtile[:, dc:dc + 2, :],
                            start=(dc == 0), stop=(dc == 2), perf_mode=DR)
                copy_eng()(o[:, lt2:lt2 + 2, :], ps[:, :, :])
            return o

        def score_phase(qT, kT, bias):
            """scores (keys on partitions) -> exp -> probsT fp8."""
            probsT = attnp.tile([P, KC, L], F8, tag="probsT8", name="probsT")
            for kt in range(KC):
                ps = psB.tile([P, 2, NH], F32, tag="scB", name="scores")
                for qh in range(2):
                    for ac in (0, 2):
                        nc.tensor.matmul(
                            ps[:, qh, :],
                            kT[:, ac:ac + 2, kt * P:(kt + 1) * P],
                            qT[:, ac:ac + 2, qh * NH:(qh + 1) * NH],
                            start=(ac == 0), stop=(ac == 2), perf_mode=DR)
                nc.scalar.activation(
                    probsT[:, kt, :].rearrange("p (h x) -> p h x", h=2),
                    ps[:, :, :], EXP, scale=SSCALE, bias=bias)
            return probsT

        def sums_phase(probsTs):
            """key-sums per query via ones-matmul, then reciprocal in
            natural [1, L] layout (cheap on DVE, early in its queue)."""
            rc_nats = []
            for i, probsT in enumerate(probsTs):
                sums_sb = smallp.tile([1, L], F32, tag=f"sums_sb{i}",
                                      name="sums_sb", bufs=2)
                st = psB.tile([P, 2, NH], F32, tag="scB", name="sums")
                for qh in range(2):
                    for ktp in (0, 2, 4, 6):
                        nc.tensor.matmul(
                            st[0:1, qh, :],
                            ones8[:, ktp:ktp + 2, 0:1],
                            probsT[:, ktp:ktp + 2, qh * NH:(qh + 1) * NH],
                            start=(ktp == 0), stop=(ktp == 6), perf_mode=DR)
                nc.scalar.copy(
                    sums_sb[0:1, :].rearrange("o (h x) -> o h x", h=2),
                    st[0:1, :, :])
                rc_nat = smallp.tile([1, L], F32, tag=f"rc_nat{i}",
                                     name="rc_nat", bufs=2)
                nc.vector.reciprocal(rc_nat[0:1, :], sums_sb[0:1, :])
                rc_nats.append(rc_nat)
            return rc_nats

        def pv_phase(attns, rc_nats):
            """PV + writers for the stage's 1-2 attentions. attns is a list
            of (probsT, v, writer). The natural-layout reciprocals are
            PE-transposed into [P, n*LT] (one accumulation group, one bank)."""
            n = len(attns)

            def pv_pair(probsT, v, qt2):
                po = psA.tile([P, 2, A], F32, tag="psA", name="ps_pv")
                for j in range(2):
                    qt = qt2 + j
                    for kc in (0, 2, 4, 6):
                        nc.tensor.matmul(
                            po[:, j, :],
                            probsT[:, kc:kc + 2, qt * P:(qt + 1) * P],
                            v[:, kc:kc + 2, :],
                            start=(kc == 0), stop=(kc == 6), perf_mode=DR)
                return po

            p1, v1, w1 = attns[0]
            po0 = pv_pair(p1, v1, 0)
            # transpose the reciprocal rows: [1, 128] -> [128, 1] per qt,
            # one PSUM bank, single accumulation group (start only on first)
            rcTp = psB.tile([P, 2, NH], F32, tag="scB", name="rcTp")
            for i in range(n):
                for qt in range(LT):
                    nc.tensor.matmul(
                        rcTp[:, 0, i * LT + qt:i * LT + qt + 1],
                        rc_nats[i][0:1, qt * P:(qt + 1) * P], idn[0:1, 0:1],
                        start=(i == 0 and qt == 0),
                        stop=(i == n - 1 and qt == LT - 1),
                        is_transpose=True, skip_group_check=True)
            rcT = smallp.tile([P, 2 * LT], F32, tag="rcT", name="rcT")
            nc.scalar.copy(rcT[:, 0:n * LT], rcTp[:, 0, 0:n * LT])

            def rc(i, qt):
                return rcT[:, i * LT + qt:i * LT + qt + 1]

            for j in range(2):
                w1(j, po0[:, j, :], rc(0, j))
            for qt2 in range(2, LT, 2):
                po = pv_pair(p1, v1, qt2)
                for j in range(2):
                    w1(qt2 + j, po[:, j, :], rc(0, qt2 + j))
            for i in range(1, n):
                p2, v2, w2 = attns[i]
                for qt2 in range(0, LT, 2):
                    po = pv_pair(p2, v2, qt2)
                    for j in range(2):
                        w2(qt2 + j, po[:, j, :], rc(i, qt2 + j))

        blocks = [(0, "txt", "au", 0), (1, "vi", "au", 2), (2, "txt", "vi", 1)]
        stages = [("sym", blk, b, n1, n2, col)
                  for blk, n1, n2, col in blocks for b in range(BLOC)]
        stages += [("cross", b) for b in range(BLOC)]
        NS = len(stages)
        st = [dict() for _ in range(NS)]

        def emit_loads(si):
            sg = stages[si]
            par = si % 2
            if sg[0] == "sym":
                _, blk, b, n1, n2, col = sg
                if b == 0:
                    st[si]["w"] = [load_w8(blk * 4 + j, f"{blk % 2}_{j}")
                                   for j in range(4)]
                else:
                    st[si]["w"] = st[si - 1]["w"]
                st[si]["m1T"] = load_mt8(n1, b, 1, par)
                st[si]["m2T"] = load_mt8(n2, b, 2, par)
            else:
                _, b = sg
                if b == 0:
                    st[si]["w"] = [load_w8(12 + j, f"c_{j}") for j in range(2)]
                else:
                    st[si]["w"] = st[si - 1]["w"]
                st[si]["xT"] = load_mt8("txt", b, 1, par)

        def emit_proj_k(si):
            sg = stages[si]
            par = si % 2
            d = st[si]
            if sg[0] == "sym":
                w = d["w"]
                # fused: scoresT_1 = (m1 G1) @ m2^T, scoresT_2 = (m2 G2) @ m1^T
                d["k1T"] = proj_T(w[0], d["m1T"], f"k1T{par}")
                d["k2T"] = proj_T(w[1], d["m2T"], f"k2T{par}")
                d["q2T"] = d["m2T"]
                d["q1T"] = d["m1T"]
            else:
                _, b = sg
                w = d["w"]
                # one bulk transpose of av from DRAM, then cast to fp8
                nc.sync.dma_start_transpose(out=avT[b][:, :, :], in_=avscr[b])
                nc.vector.tensor_copy(avT8[b][:, :, :], avT[b][:, :, :])
                d["k1T"] = proj_T(w[0], d["xT"], f"k1T{par}")
                d["q2T"] = avT8[b]

        def emit_proj_v(si):
            sg = stages[si]
            par = si % 2
            d = st[si]
            if sg[0] == "sym":
                w = d["w"]
                d["v1"] = proj_N(w[2], d["m1T"], f"v1{par}")
                d["v2"] = proj_N(w[3], d["m2T"], f"v2{par}")
            else:
                w = d["w"]
                d["v1"] = proj_N(w[1], d["xT"], f"v1{par}")

        def emit_scores(si):
            sg = stages[si]
            d = st[si]
            if sg[0] == "sym":
                d["p1"] = score_phase(d["q2T"], d["k1T"], ebias[:, 0:1])
                d["p2"] = score_phase(d["q1T"], d["k2T"], ebias[:, 0:1])
            else:
                d["p1"] = score_phase(d["q2T"], d["k1T"], cbias[:, 0:1])

        def emit_sums(si):
            d = st[si]
            ps = [d["p1"]] + ([d["p2"]] if "p2" in d else [])
            d["rc_nat"] = sums_phase(ps)

        def emit_pv(si):
            sg = stages[si]
            d = st[si]
            if sg[0] == "sym":
                _, blk, b, n1, n2, col = sg
                o1r = projp.tile([P, LT, A], F32, tag="o1r", name="o1r")

                def writer1(qt, po, rc, blk=blk, b=b):
                    res_t = smallp.tile([P, A], F32, tag="res_t", name="res_t")
                    nc.sync.dma_start(
                        out=res_t[:, :],
                        in_=res[blk, b, qt * P:(qt + 1) * P, :])
                    nc.vector.scalar_tensor_tensor(
                        o1r[:, qt, :], po, rc, res_t[:, :], op0=MULT, op1=ADD)

                def writer2(qt, po, rc, blk=blk, b=b, col=col):
                    out_t = smallp.tile([P, A], F32, tag="out_t", name="out_t")
                    nc.vector.scalar_tensor_tensor(
                        out_t[:, :], po, rc, o1r[:, qt, :], op0=MULT, op1=ADD)
                    nc.sync.dma_start(
                        out=out[b, qt * P:(qt + 1) * P, col * A:(col + 1) * A],
                        in_=out_t[:, :])
                    if blk == 1:
                        av_bf = smallp.tile([P, A], BF16, tag="av_bf",
                                            name="av_bf", bufs=8)
                        nc.vector.tensor_copy(av_bf[:, :], out_t[:, :])
                        nc.sync.dma_start(
                            out=avscr[b, qt * P:(qt + 1) * P, :],
                            in_=av_bf[:, :])

                pv_phase([(d["p1"], d["v1"], writer1),
                          (d["p2"], d["v2"], writer2)], d["rc_nat"])
            else:
                _, b = sg

                def writer_c(qt, po, rc, b=b):
                    out_t = smallp.tile([P, A], F32, tag="out_t", name="out_tc")
                    nc.scalar.activation(out_t[:, :], po, COPY, scale=rc)
                    nc.sync.dma_start(
                        out=out[b, qt * P:(qt + 1) * P, 3 * A:4 * A],
                        in_=out_t[:, :])

                pv_phase([(d["p1"], d["v1"], writer_c)], d["rc_nat"])

        # software pipeline: stage s+1's projections are emitted between
        # stage s's scores and PV; the sums matmuls slot between the k- and
        # v-projections so they never wait on the trailing exps
        emit_loads(0)
        emit_loads(1)
        emit_proj_k(0)
        emit_proj_v(0)
        for si in range(NS):
            emit_scores(si)
            if si + 2 < NS:
                emit_loads(si + 2)
            if si + 1 < NS:
                emit_proj_k(si + 1)
            emit_sums(si)
            if si + 1 < NS:
                emit_proj_v(si + 1)
            emit_pv(si)


_nc_cache = None
last_results = None


def _get_nc():
    global _nc_cache
    if _nc_cache is None:
        _nc_cache = _build()
    return _nc_cache


def kernel(**inputs):
    global last_results
    txt = np.asarray(inputs["txt"], dtype=np.float32)
    au = np.asarray(inputs["au"], dtype=np.float32)
    vi = np.asarray(inputs["vi"], dtype=np.float32)

    nat = {"txt": txt, "au": au, "vi": vi}
    mt8 = {n: np.ascontiguousarray(v.transpose(0, 2, 1)).astype(ml_dtypes.float8_e4m3)
           for n, v in nat.items()}
    W = {n: np.asarray(inputs[n], dtype=np.float32) for n in W_NAMES}
    wlist = []
    for blk in ("ta", "va", "tv"):
        wlist.append(GS * (W[f"{blk}_kx"].T @ W[f"{blk}_qy"]))  # G1
        wlist.append(GS * (W[f"{blk}_ky"].T @ W[f"{blk}_qx"]))  # G2
        wlist.append(W[f"{blk}_vx"].T)
        wlist.append(W[f"{blk}_vy"].T)
    wlist.append(GS * (W["tav_k"].T @ W["tav_q"]))  # Gc
    wlist.append(W["tav_v"].T)
    wt8_all = np.ascontiguousarray(np.stack(wlist)).astype(ml_dtypes.float8_e4m3)
    res_all = np.stack([txt + au, vi + au, txt + vi])

    in_maps = []
    for c in range(NCORES):
        sl = slice(c * BLOC, (c + 1) * BLOC)
        in_maps.append({
            "mt_txt": mt8["txt"][sl],
            "mt_au": mt8["au"][sl],
            "mt_vi": mt8["vi"][sl],
            "res": np.ascontiguousarray(res_all[:, sl]),
            "wt8": wt8_all,
            "ident128": np.eye(P, dtype=np.float32),
        })

    nc = _get_nc()
    last_results = run_bass_kernel_spmd(nc, in_maps, core_ids=list(range(NCORES)))
    core_out = np.concatenate(
        [np.asarray(last_results.results[c]["out"]) for c in range(NCORES)], axis=0)
    return np.concatenate([txt, au, vi, core_out], axis=-1).astype(np.float32)


# revision 24
# speedup vs baseline: 1.0562x; 1.0562x over previous
"""v3: all-fp8 (e4m3) DoubleRow matmuls for all 7 attentions; transposed
scores (keys on partitions, no probability transposes); softmax sums via
ones-matmul into PSUM + PE transpose of the [1,L] sums row; interleaved
score/PV phases of the two symmetric attentions to hide exp latency.
Sim-predicted rel err ~7.6e-3 (gate 2e-2)."""

import numpy as np
import ml_dtypes

from concourse import bacc, bass, tile, mybir
from concourse.bass_utils import run_bass_kernel_spmd

B, L, D = 16, 1024, 512
A = D
NCORES = 8
BLOC = B // NCORES
P = 128
DC = D // P
AC = A // P
LT = L // P
KC = L // P
NH = 512
SCALE = float(1.0 / np.sqrt(np.float32(D)))
GS = 8.0  # host-side scale on the fused G = Wk^T @ Wq matrices
SSCALE = SCALE / GS
EXP_BIAS = -1.0   # symmetric attns: keeps exp() under fp8e4 max 240
CROSS_BIAS = -5.0  # cross attn has wider score range (queries = av)

F32 = mybir.dt.float32
BF16 = mybir.dt.bfloat16
F8 = mybir.dt.float8e4
DR = mybir.MatmulPerfMode.DoubleRow
EXP = mybir.ActivationFunctionType.Exp
COPY = mybir.ActivationFunctionType.Copy
MULT = mybir.AluOpType.mult
ADD = mybir.AluOpType.add

W_NAMES = [f"{blk}_{w}" for blk in ("ta", "va", "tv")
           for w in ("kx", "qx", "vx", "ky", "qy", "vy")] + [
    "tav_k", "tav_q", "tav_v"]


def _build():
    nc = bacc.Bacc("TRN2", target_bir_lowering=False, debug=False,
                   num_devices=NCORES)

    mt_txt = nc.dram_tensor("mt_txt", (BLOC, D, L), F8, kind="ExternalInput").ap()
    mt_au = nc.dram_tensor("mt_au", (BLOC, D, L), F8, kind="ExternalInput").ap()
    mt_vi = nc.dram_tensor("mt_vi", (BLOC, D, L), F8, kind="ExternalInput").ap()
    res = nc.dram_tensor("res", (3, BLOC, L, D), F32, kind="ExternalInput").ap()
    wt8 = nc.dram_tensor("wt8", (14, D, A), F8, kind="ExternalInput").ap()
    ident128 = nc.dram_tensor("ident128", (P, P), F32, kind="ExternalInput").ap()
    out = nc.dram_tensor("out", (BLOC, L, 4 * A), F32, kind="ExternalOutput").ap()
    avscr = nc.dram_tensor("avscr", (BLOC, L, A), BF16, kind="ExternalOutput").ap()

    with tile.TileContext(nc) as tc:
        _body(nc, tc, mt_txt, mt_au, mt_vi, res, wt8, ident128, out, avscr)

    nc.compile()
    return nc


def _body(nc, tc, mt_txt, mt_au, mt_vi, res, wt8, ident128, out, avscr):
    mt_dram = {"txt": mt_txt, "au": mt_au, "vi": mt_vi}

    with (
        tc.tile_pool(name="persist", bufs=1) as persist,
        tc.tile_pool(name="wpool", bufs=1) as wpool,
        tc.tile_pool(name="mpool", bufs=1) as mpool,
        tc.tile_pool(name="proj", bufs=1) as projp,
        tc.tile_pool(name="attn", bufs=2) as attnp,
        tc.tile_pool(name="small", bufs=3) as smallp,
        tc.tile_pool(name="ps_a", bufs=2, space=bass.MemorySpace.PSUM) as psA,
        tc.tile_pool(name="ps_b", bufs=2, space=bass.MemorySpace.PSUM) as psB,
    ):
        avT = [persist.tile([P, AC, L], BF16, tag=f"avT{b}", name=f"avT{b}")
               for b in range(BLOC)]
        avT8 = [persist.tile([P, AC, L], F8, tag=f"avT8{b}", name=f"avT8{b}")
                for b in range(BLOC)]
        # pad rows to 32B so the DoubleRow dual-fp8 ldweights stride is legal
        ones8 = persist.tile([P, KC, 32], F8, tag="ones8", name="ones8")
        nc.gpsimd.memset(ones8[:, :, :], 1.0)
        ebias = persist.tile([P, 1], F32, tag="ebias", name="ebias")
        nc.gpsimd.memset(ebias[:, :], EXP_BIAS)
        cbias = persist.tile([P, 1], F32, tag="cbias", name="cbias")
        nc.gpsimd.memset(cbias[:, :], CROSS_BIAS)
        idn = persist.tile([P, P], F32, tag="idn", name="idn")
        nc.sync.dma_start(out=idn[:, :], in_=ident128)

        # all PSUM->SBUF projection copies on DVE: the scalar queue stays
        # shallow so exps and sums copies (which gate PV) retire promptly
        def copy_eng():
            return nc.vector.tensor_copy

        def load_w8(j, slot):
            t = wpool.tile([P, DC, A], F8, tag=f"w{slot}", name=f"w{j}")
            nc.sync.dma_start(out=t[:, :, :],
                              in_=wt8[j].rearrange("(dc p) a -> p dc a", p=P))
            return t

        def load_mt8(name, b, slot, par):
            t = mpool.tile([P, DC, L], F8, tag=f"mT{slot}_{b}_{par}",
                           name=f"mT_{name}{b}")
            src = mt_dram[name][b].rearrange("(dc p) l -> p dc l", p=P)
            nc.sync.dma_start(out=t[:, 0:2, :], in_=src[:, 0:2, :])
            nc.sync.dma_start(out=t[:, 2:4, :], in_=src[:, 2:4, :])
            return t

        def proj_T(wtile, mtile, tag):
            o = projp.tile([P, AC, L], F8, tag=tag, name=tag)
            for ac in range(AC):
                ps = psA.tile([P, 2, NH], F32, tag="psA", name="ps_pt")
                for h in range(2):
                    for dc in (0, 2):
                        nc.tensor.matmul(
                            ps[:, h, :],
                            wtile[:, dc:dc + 2, ac * P:(ac + 1) * P],
                            mtile[:, dc:dc + 2, h * NH:(h + 1) * NH],
                            start=(dc == 0), stop=(dc == 2), perf_mode=DR)
                copy_eng()(
                    o[:, ac, :].rearrange("p (h x) -> p h x", h=2), ps[:, :, :])
            return o

        def proj_N(wtile, mtile, tag):
            o = projp.tile([P, KC, A], F8, tag=tag, name=tag)
            for lt2 in range(0, LT, 2):
                ps = psA.tile([P, 2, A], F32, tag="psA", name="ps_pn")
                for j in range(2):
                    lt = lt2 + j
                    for dc in (0, 2):
                        nc.tensor.matmul(
                            ps[:, j, :],
                            mtile[:, dc:dc + 2, lt * P:(lt + 1) * P],
                            wtile[:, dc:dc + 2, :],
                            start=(dc == 0), stop=(dc == 2), perf_mode=DR)
                copy_eng()(o[:, lt2:lt2 + 2, :], ps[:, :, :])
            return o

        def score_phase(qT, kT, bias):
            """scores (keys on partitions) -> exp -> probsT fp8."""
            probsT = attnp.tile([P, KC, L], F8, tag="probsT8", name="probsT")
            for kt in range(KC):
                ps = psB.tile([P, 2, NH], F32, tag="scB", name="scores")
                for qh in range(2):
                    for ac in (0, 2):
                        nc.tensor.matmul(
                            ps[:, qh, :],
                            kT[:, ac:ac + 2, kt * P:(kt + 1) * P],
                            qT[:, ac:ac + 2, qh * NH:(qh + 1) * NH],
                            start=(ac == 0), stop=(ac == 2), perf_mode=DR)
                nc.scalar.activation(
                    probsT[:, kt, :].rearrange("p (h x) -> p h x", h=2),
                    ps[:, :, :], EXP, scale=SSCALE, bias=bias)
            return probsT

        def sums_phase(probsTs):
            """key-sums per query via ones-matmul, then reciprocal in
            natural [1, L] layout (cheap on DVE, early in its queue)."""
            rc_nats = []
            for i, probsT in enumerate(probsTs):
                sums_sb = smallp.tile([1, L], F32, tag=f"sums_sb{i}",
                                      name="sums_sb", bufs=2)
                st = psB.tile([P, 2, NH], F32, tag="scB", name="sums")
                for qh in range(2):
                    for ktp in (0, 2, 4, 6):
                        nc.tensor.matmul(
                            st[0:1, qh, :],
                            ones8[:, ktp:ktp + 2, 0:1],
                            probsT[:, ktp:ktp + 2, qh * NH:(qh + 1) * NH],
                            start=(ktp == 0), stop=(ktp == 6), perf_mode=DR)
                nc.scalar.copy(
                    sums_sb[0:1, :].rearrange("o (h x) -> o h x", h=2),
                    st[0:1, :, :])
                rc_nats.append(sums_sb)
            return rc_nats

        def pv_phase(attns, rc_nats):
            """PV + writers for the stage's 1-2 attentions. attns is a list
            of (probsT, v, writer). The natural-layout reciprocals are
            PE-transposed into [P, n*LT] (one accumulation group, one bank)."""
            n = len(attns)

            def pv_pair(probsT, v, qt2):
                po = psA.tile([P, 2, A], F32, tag="psA", name="ps_pv")
                for j in range(2):
                    qt = qt2 + j
                    for kc in (0, 2, 4, 6):
                        nc.tensor.matmul(
                            po[:, j, :],
                            probsT[:, kc:kc + 2, qt * P:(qt + 1) * P],
                            v[:, kc:kc + 2, :],
                            start=(kc == 0), stop=(kc == 6), perf_mode=DR)
                return po

            p1, v1, w1 = attns[0]
            po0 = pv_pair(p1, v1, 0)
            # transpose the reciprocal rows: [1, 128] -> [128, 1] per qt,
            # one PSUM bank, single accumulation group (start only on first)
            rcTp = psB.tile([P, 2, NH], F32, tag="scB", name="rcTp")
            for i in range(n):
                for qt in range(LT):
                    nc.tensor.matmul(
                        rcTp[:, 0, i * LT + qt:i * LT + qt + 1],
                        rc_nats[i][0:1, qt * P:(qt + 1) * P], idn[0:1, 0:1],
                        start=(i == 0 and qt == 0),
                        stop=(i == n - 1 and qt == LT - 1),
                        is_transpose=True, skip_group_check=True)
            rcT = smallp.tile([P, 2 * LT], F32, tag="rcT", name="rcT")
            nc.vector.reciprocal(rcT[:, 0:n * LT], rcTp[:, 0, 0:n * LT])

            def rc(i, qt):
                return rcT[:, i * LT + qt:i * LT + qt + 1]

            for j in range(2):
                w1(j, po0[:, j, :], rc(0, j))
            for qt2 in range(2, LT, 2):
                po = pv_pair(p1, v1, qt2)
                for j in range(2):
                    w1(qt2 + j, po[:, j, :], rc(0, qt2 + j))
            for i in range(1, n):
                p2, v2, w2 = attns[i]
                for qt2 in range(0, LT, 2):
                    po = pv_pair(p2, v2, qt2)
                    for j in range(2):
                        w2(qt2 + j, po[:, j, :], rc(i, qt2 + j))

        blocks = [(0, "txt", "au", 0), (1, "vi", "au", 2), (2, "txt", "vi", 1)]
        stages = [("sym", blk, b, n1, n2, col)
                  for blk, n1, n2, col in blocks for b in range(BLOC)]
        stages += [("cross", b) for b in range(BLOC)]
        NS = len(stages)
        st = [dict() for _ in range(NS)]

        def emit_loads(si):
            sg = stages[si]
            par = si % 2
            if sg[0] == "sym":
                _, blk, b, n1, n2, col = sg
                if b == 0:
                    st[si]["w"] = [load_w8(blk * 4 + j, f"{blk % 2}_{j}")
                                   for j in range(4)]
                else:
                    st[si]["w"] = st[si - 1]["w"]
                st[si]["m1T"] = load_mt8(n1, b, 1, par)
                st[si]["m2T"] = load_mt8(n2, b, 2, par)
            else:
                _, b = sg
                if b == 0:
                    st[si]["w"] = [load_w8(12 + j, f"c_{j}") for j in range(2)]
                else:
                    st[si]["w"] = st[si - 1]["w"]
                st[si]["xT"] = load_mt8("txt", b, 1, par)

        def emit_proj_k(si):
            sg = stages[si]
            par = si % 2
            d = st[si]
            if sg[0] == "sym":
                w = d["w"]
                # fused: scoresT_1 = (m1 G1) @ m2^T, scoresT_2 = (m2 G2) @ m1^T
                d["k1T"] = proj_T(w[0], d["m1T"], f"k1T{par}")
                d["k2T"] = proj_T(w[1], d["m2T"], f"k2T{par}")
                d["q2T"] = d["m2T"]
                d["q1T"] = d["m1T"]
            else:
                _, b = sg
                w = d["w"]
                # one bulk transpose of av from DRAM, then cast to fp8
                nc.sync.dma_start_transpose(out=avT[b][:, :, :], in_=avscr[b])
                nc.vector.tensor_copy(avT8[b][:, :, :], avT[b][:, :, :])
                d["k1T"] = proj_T(w[0], d["xT"], f"k1T{par}")
                d["q2T"] = avT8[b]

        def emit_proj_v(si):
            sg = stages[si]
            par = si % 2
            d = st[si]
            if sg[0] == "sym":
                w = d["w"]
                d["v1"] = proj_N(w[2], d["m1T"], f"v1{par}")
                d["v2"] = proj_N(w[3], d["m2T"], f"v2{par}")
            else:
                w = d["w"]
                d["v1"] = proj_N(w[1], d["xT"], f"v1{par}")

        def emit_scores(si):
            sg = stages[si]
            d = st[si]
            if sg[0] == "sym":
                d["p1"] = score_phase(d["q2T"], d["k1T"], ebias[:, 0:1])
                d["p2"] = score_phase(d["q1T"], d["k2T"], ebias[:, 0:1])
            else:
                d["p1"] = score_phase(d["q2T"], d["k1T"], cbias[:, 0:1])

        def emit_sums(si):
            d = st[si]
            ps = [d["p1"]] + ([d["p2"]] if "p2" in d else [])
            d["rc_nat"] = sums_phase(ps)

        def emit_pv(si):
            sg = stages[si]
            d = st[si]
            if sg[0] == "sym":
                _, blk, b, n1, n2, col = sg
                o1r = projp.tile([P, LT, A], F32, tag="o1r", name="o1r")

                def writer1(qt, po, rc, blk=blk, b=b):
                    res_t = smallp.tile([P, A], F32, tag="res_t", name="res_t")
                    nc.sync.dma_start(
                        out=res_t[:, :],
                        in_=res[blk, b, qt * P:(qt + 1) * P, :])
                    nc.vector.scalar_tensor_tensor(
                        o1r[:, qt, :], po, rc, res_t[:, :], op0=MULT, op1=ADD)

                def writer2(qt, po, rc, blk=blk, b=b, col=col):
                    out_t = smallp.tile([P, A], F32, tag="out_t", name="out_t")
                    nc.vector.scalar_tensor_tensor(
                        out_t[:, :], po, rc, o1r[:, qt, :], op0=MULT, op1=ADD)
                    nc.sync.dma_start(
                        out=out[b, qt * P:(qt + 1) * P, col * A:(col + 1) * A],
                        in_=out_t[:, :])
                    if blk == 1:
                        av_bf = smallp.tile([P, A], BF16, tag="av_bf",
                                            name="av_bf", bufs=8)
                        nc.vector.tensor_copy(av_bf[:, :], out_t[:, :])
                        nc.sync.dma_start(
                            out=avscr[b, qt * P:(qt + 1) * P, :],
                            in_=av_bf[:, :])

                pv_phase([(d["p1"], d["v1"], writer1),
                          (d["p2"], d["v2"], writer2)], d["rc_nat"])
            else:
                _, b = sg

                def writer_c(qt, po, rc, b=b):
                    out_t = smallp.tile([P, A], F32, tag="out_t", name="out_tc")
                    nc.scalar.activation(out_t[:, :], po, COPY, scale=rc)
                    nc.sync.dma_start(
                        out=out[b, qt * P:(qt + 1) * P, 3 * A:4 * A],
                        in_=out_t[:, :])

                pv_phase([(d["p1"], d["v1"], writer_c)], d["rc_nat"])

        # software pipeline: stage s+1's projections are emitted between
        # stage s's scores and PV; the sums matmuls slot between the k- and
        # v-projections so they never wait on the trailing exps
        emit_loads(0)
        emit_loads(1)
        emit_proj_k(0)
        emit_proj_v(0)
        for si in range(NS):
            emit_scores(si)
            if si + 2 < NS:
                emit_loads(si + 2)
            if si + 1 < NS:
                emit_proj_k(si + 1)
            emit_sums(si)
            if si + 1 < NS:
                emit_proj_v(si + 1)
            emit_pv(si)


_nc_cache = None
last_results = None


def _get_nc():
    global _nc_cache
    if _nc_cache is None:
        _nc_cache = _build()
    return _nc_cache


def kernel(**inputs):
    global last_results
    txt = np.asarray(inputs["txt"], dtype=np.float32)
    au = np.asarray(inputs["au"], dtype=np.float32)
    vi = np.asarray(inputs["vi"], dtype=np.float32)

    nat = {"txt": txt, "au": au, "vi": vi}
    mt8 = {n: np.ascontiguousarray(v.transpose(0, 2, 1)).astype(ml_dtypes.float8_e4m3)
           for n, v in nat.items()}
    W = {n: np.asarray(inputs[n], dtype=np.float32) for n in W_NAMES}
    wlist = []
    for blk in ("ta", "va", "tv"):
        wlist.append(GS * (W[f"{blk}_kx"].T @ W[f"{blk}_qy"]))  # G1
        wlist.append(GS * (W[f"{blk}_ky"].T @ W[f"{blk}_qx"]))  # G2
        wlist.append(W[f"{blk}_vx"].T)
        wlist.append(W[f"{blk}_vy"].T)
    wlist.append(GS * (W["tav_k"].T @ W["tav_q"]))  # Gc
    wlist.append(W["tav_v"].T)
    wt8_all = np.ascontiguousarray(np.stack(wlist)).astype(ml_dtypes.float8_e4m3)
    res_all = np.stack([txt + au, vi + au, txt + vi])

    in_maps = []
    for c in range(NCORES):
        sl = slice(c * BLOC, (c + 1) * BLOC)
        in_maps.append({
            "mt_txt": mt8["txt"][sl],
            "mt_au": mt8["au"][sl],
            "mt_vi": mt8["vi"][sl],
            "res": np.ascontiguousarray(res_all[:, sl]),
            "wt8": wt8_all,
            "ident128": np.eye(P, dtype=np.float32),
        })

    nc = _get_nc()
    last_results = run_bass_kernel_spmd(nc, in_maps, core_ids=list(range(NCORES)))
    core_out = np.concatenate(
        [np.asarray(last_results.results[c]["out"]) for c in range(NCORES)], axis=0)
    return np.concatenate([txt, au, vi, core_out], axis=-1).astype(np.float32)


# revision 25
# speedup vs baseline: 1.2500x; 1.1835x over previous
"""fp8 (e4m3) AttentionFuser kernel, data-parallel over batch on 8 cores.

Key structure:
- Fused QK projections: scoresT = (m1 @ G) @ m2^T with G = Wk^T Wq
  precomputed on host (x8 scale folded into the exp), so each attention
  needs one projection instead of two and the raw input tile is the
  scores moving operand.
- All matmuls fp8 DoubleRow (256-deep contraction per instruction).
- Transposed scores (keys on partitions): probabilities come out of the
  exp already in PV layout; no probability transposes anywhere.
- Softmax normalization: key-sums via ones-matmul into one PSUM bank,
  PE-transposed [1,128]->[128,1] per query tile, reciprocal on DVE,
  applied as the per-partition scale of the fused (po*rc)+residual
  scalar_tensor_tensor writers.
- 8-stage software pipeline (6 symmetric + 2 cross stages): stage s+1's
  projections are emitted between stage s's scores and PV; the sums
  matmuls slot between the k- and v-projections so nothing waits on
  trailing exps. av for the cross attention is spilled to DRAM in bf16
  and bulk-DMA-transposed once per batch.
Measured: ~390-460us on 8 trn2 cores (throttle-dependent), rel err 4.6e-3."""

import numpy as np
import ml_dtypes

from concourse import bacc, bass, tile, mybir
from concourse.bass_utils import run_bass_kernel_spmd

B, L, D = 16, 1024, 512
A = D
NCORES = 8
BLOC = B // NCORES
P = 128
DC = D // P
AC = A // P
LT = L // P
KC = L // P
NH = 512
SCALE = float(1.0 / np.sqrt(np.float32(D)))
GS = 8.0  # host-side scale on the fused G = Wk^T @ Wq matrices
SSCALE = SCALE / GS
EXP_BIAS = -1.0   # symmetric attns: keeps exp() under fp8e4 max 240
CROSS_BIAS = -5.0  # cross attn has wider score range (queries = av)

F32 = mybir.dt.float32
BF16 = mybir.dt.bfloat16
F8 = mybir.dt.float8e4
DR = mybir.MatmulPerfMode.DoubleRow
EXP = mybir.ActivationFunctionType.Exp
COPY = mybir.ActivationFunctionType.Copy
MULT = mybir.AluOpType.mult
ADD = mybir.AluOpType.add

W_NAMES = [f"{blk}_{w}" for blk in ("ta", "va", "tv")
           for w in ("kx", "qx", "vx", "ky", "qy", "vy")] + [
    "tav_k", "tav_q", "tav_v"]


def _build():
    nc = bacc.Bacc("TRN2", target_bir_lowering=False, debug=False,
                   num_devices=NCORES)

    mt_txt = nc.dram_tensor("mt_txt", (BLOC, D, L), F8, kind="ExternalInput").ap()
    mt_au = nc.dram_tensor("mt_au", (BLOC, D, L), F8, kind="ExternalInput").ap()
    mt_vi = nc.dram_tensor("mt_vi", (BLOC, D, L), F8, kind="ExternalInput").ap()
    res = nc.dram_tensor("res", (3, BLOC, L, D), F32, kind="ExternalInput").ap()
    wt8 = nc.dram_tensor("wt8", (14, D, A), F8, kind="ExternalInput").ap()
    ident128 = nc.dram_tensor("ident128", (P, P), F32, kind="ExternalInput").ap()
    out = nc.dram_tensor("out", (BLOC, L, 4 * A), F32, kind="ExternalOutput").ap()
    avscr = nc.dram_tensor("avscr", (BLOC, L, A), BF16, kind="ExternalOutput").ap()

    with tile.TileContext(nc) as tc:
        _body(nc, tc, mt_txt, mt_au, mt_vi, res, wt8, ident128, out, avscr)

    nc.compile()
    return nc


def _body(nc, tc, mt_txt, mt_au, mt_vi, res, wt8, ident128, out, avscr):
    mt_dram = {"txt": mt_txt, "au": mt_au, "vi": mt_vi}

    with (
        tc.tile_pool(name="persist", bufs=1) as persist,
        tc.tile_pool(name="wpool", bufs=1) as wpool,
        tc.tile_pool(name="mpool", bufs=1) as mpool,
        tc.tile_pool(name="proj", bufs=1) as projp,
        tc.tile_pool(name="attn", bufs=2) as attnp,
        tc.tile_pool(name="small", bufs=3) as smallp,
        tc.tile_pool(name="ps_a", bufs=2, space=bass.MemorySpace.PSUM) as psA,
        tc.tile_pool(name="ps_b", bufs=2, space=bass.MemorySpace.PSUM) as psB,
    ):
        avT = [persist.tile([P, AC, L], BF16, tag=f"avT{b}", name=f"avT{b}")
               for b in range(BLOC)]
        avT8 = [persist.tile([P, AC, L], F8, tag=f"avT8{b}", name=f"avT8{b}")
                for b in range(BLOC)]
        # pad rows to 32B so the DoubleRow dual-fp8 ldweights stride is legal
        ones8 = persist.tile([P, KC, 32], F8, tag="ones8", name="ones8")
        nc.gpsimd.memset(ones8[:, :, :], 1.0)
        ebias = persist.tile([P, 1], F32, tag="ebias", name="ebias")
        nc.gpsimd.memset(ebias[:, :], EXP_BIAS)
        cbias = persist.tile([P, 1], F32, tag="cbias", name="cbias")
        nc.gpsimd.memset(cbias[:, :], CROSS_BIAS)
        idn = persist.tile([P, P], F32, tag="idn", name="idn")
        nc.sync.dma_start(out=idn[:, :], in_=ident128)

        # all PSUM->SBUF projection copies on DVE: the scalar queue stays
        # shallow so exps and sums copies (which gate PV) retire promptly
        def copy_eng():
            return nc.vector.tensor_copy

        def load_w8(j, slot):
            t = wpool.tile([P, DC, A], F8, tag=f"w{slot}", name=f"w{j}")
            nc.sync.dma_start(out=t[:, :, :],
                              in_=wt8[j].rearrange("(dc p) a -> p dc a", p=P))
            return t

        def load_mt8(name, b, slot, par):
            t = mpool.tile([P, DC, L], F8, tag=f"mT{slot}_{b}_{par}",
                           name=f"mT_{name}{b}")
            src = mt_dram[name][b].rearrange("(dc p) l -> p dc l", p=P)
            nc.sync.dma_start(out=t[:, 0:2, :], in_=src[:, 0:2, :])
            nc.sync.dma_start(out=t[:, 2:4, :], in_=src[:, 2:4, :])
            return t

        def proj_T(wtile, mtile, tag):
            o = projp.tile([P, AC, L], F8, tag=tag, name=tag)
            for ac in range(AC):
                ps = psA.tile([P, 2, NH], F32, tag="psA", name="ps_pt")
                for h in range(2):
                    for dc in (0, 2):
                        nc.tensor.matmul(
                            ps[:, h, :],
                            wtile[:, dc:dc + 2, ac * P:(ac + 1) * P],
                            mtile[:, dc:dc + 2, h * NH:(h + 1) * NH],
                            start=(dc == 0), stop=(dc == 2), perf_mode=DR)
                copy_eng()(
                    o[:, ac, :].rearrange("p (h x) -> p h x", h=2), ps[:, :, :])
            return o

        def proj_N(wtile, mtile, tag):
            o = projp.tile([P, KC, A], F8, tag=tag, name=tag)
            for lt2 in range(0, LT, 2):
                ps = psA.tile([P, 2, A], F32, tag="psA", name="ps_pn")
                for j in range(2):
                    lt = lt2 + j
                    for dc in (0, 2):
                        nc.tensor.matmul(
                            ps[:, j, :],
                            mtile[:, dc:dc + 2, lt * P:(lt + 1) * P],
                            wtile[:, dc:dc + 2, :],
                            start=(dc == 0), stop=(dc == 2), perf_mode=DR)
                copy_eng()(o[:, lt2:lt2 + 2, :], ps[:, :, :])
            return o

        def score_phase(qT, kT, bias):
            """scores (keys on partitions) -> exp -> probsT fp8."""
            probsT = attnp.tile([P, KC, L], F8, tag="probsT8", name="probsT")
            for kt in range(KC):
                ps = psB.tile([P, 2, NH], F32, tag="scB", name="scores")
                for qh in range(2):
                    for ac in (0, 2):
                        nc.tensor.matmul(
                            ps[:, qh, :],
                            kT[:, ac:ac + 2, kt * P:(kt + 1) * P],
                            qT[:, ac:ac + 2, qh * NH:(qh + 1) * NH],
                            start=(ac == 0), stop=(ac == 2), perf_mode=DR)
                nc.scalar.activation(
                    probsT[:, kt, :].rearrange("p (h x) -> p h x", h=2),
                    ps[:, :, :], EXP, scale=SSCALE, bias=bias)
            return probsT

        def sums_phase(probsTs):
            """key-sums per query via ones-matmul, then reciprocal in
            natural [1, L] layout (cheap on DVE, early in its queue)."""
            rc_nats = []
            for i, probsT in enumerate(probsTs):
                sums_sb = smallp.tile([1, L], F32, tag=f"sums_sb{i}",
                                      name="sums_sb", bufs=2)
                st = psB.tile([P, 2, NH], F32, tag="scB", name="sums")
                for qh in range(2):
                    for ktp in (0, 2, 4, 6):
                        nc.tensor.matmul(
                            st[0:1, qh, :],
                            ones8[:, ktp:ktp + 2, 0:1],
                            probsT[:, ktp:ktp + 2, qh * NH:(qh + 1) * NH],
                            start=(ktp == 0), stop=(ktp == 6), perf_mode=DR)
                nc.scalar.copy(
                    sums_sb[0:1, :].rearrange("o (h x) -> o h x", h=2),
                    st[0:1, :, :])
                rc_nats.append(sums_sb)
            return rc_nats

        def pv_phase(attns, rc_nats):
            """PV + writers for the stage's 1-2 attentions. attns is a list
            of (probsT, v, writer). The natural-layout reciprocals are
            PE-transposed into [P, n*LT] (one accumulation group, one bank)."""
            n = len(attns)

            def pv_pair(probsT, v, qt2):
                po = psA.tile([P, 2, A], F32, tag="psA", name="ps_pv")
                for j in range(2):
                    qt = qt2 + j
                    for kc in (0, 2, 4, 6):
                        nc.tensor.matmul(
                            po[:, j, :],
                            probsT[:, kc:kc + 2, qt * P:(qt + 1) * P],
                            v[:, kc:kc + 2, :],
                            start=(kc == 0), stop=(kc == 6), perf_mode=DR)
                return po

            p1, v1, w1 = attns[0]
            po0 = pv_pair(p1, v1, 0)
            # transpose the reciprocal rows: [1, 128] -> [128, 1] per qt,
            # one PSUM bank, single accumulation group (start only on first)
            rcTp = psB.tile([P, 2, NH], F32, tag="scB", name="rcTp")
            for i in range(n):
                for qt in range(LT):
                    nc.tensor.matmul(
                        rcTp[:, 0, i * LT + qt:i * LT + qt + 1],
                        rc_nats[i][0:1, qt * P:(qt + 1) * P], idn[0:1, 0:1],
                        start=(i == 0 and qt == 0),
                        stop=(i == n - 1 and qt == LT - 1),
                        is_transpose=True, skip_group_check=True)
            rcT = smallp.tile([P, 2 * LT], F32, tag="rcT", name="rcT")
            nc.vector.reciprocal(rcT[:, 0:n * LT], rcTp[:, 0, 0:n * LT])

            def rc(i, qt):
                return rcT[:, i * LT + qt:i * LT + qt + 1]

            for j in range(2):
                w1(j, po0[:, j, :], rc(0, j))
            for qt2 in range(2, LT, 2):
                po = pv_pair(p1, v1, qt2)
                for j in range(2):
                    w1(qt2 + j, po[:, j, :], rc(0, qt2 + j))
            for i in range(1, n):
                p2, v2, w2 = attns[i]
                for qt2 in range(0, LT, 2):
                    po = pv_pair(p2, v2, qt2)
                    for j in range(2):
                        w2(qt2 + j, po[:, j, :], rc(i, qt2 + j))

        blocks = [(0, "txt", "au", 0), (1, "vi", "au", 2), (2, "txt", "vi", 1)]
        stages = [("sym", blk, b, n1, n2, col)
                  for blk, n1, n2, col in blocks for b in range(BLOC)]
        stages += [("cross", b) for b in range(BLOC)]
        NS = len(stages)
        st = [dict() for _ in range(NS)]

        def emit_loads(si):
            sg = stages[si]
            par = si % 2
            if sg[0] == "sym":
                _, blk, b, n1, n2, col = sg
                if b == 0:
                    st[si]["w"] = [load_w8(blk * 4 + j, f"{blk % 2}_{j}")
                                   for j in range(4)]
                else:
                    st[si]["w"] = st[si - 1]["w"]
                st[si]["m1T"] = load_mt8(n1, b, 1, par)
                st[si]["m2T"] = load_mt8(n2, b, 2, par)
            else:
                _, b = sg
                if b == 0:
                    st[si]["w"] = [load_w8(12 + j, f"c_{j}") for j in range(2)]
                else:
                    st[si]["w"] = st[si - 1]["w"]
                st[si]["xT"] = load_mt8("txt", b, 1, par)

        def emit_proj_k(si):
            sg = stages[si]
            par = si % 2
            d = st[si]
            if sg[0] == "sym":
                w = d["w"]
                # fused: scoresT_1 = (m1 G1) @ m2^T, scoresT_2 = (m2 G2) @ m1^T
                d["k1T"] = proj_T(w[0], d["m1T"], f"k1T{par}")
                d["k2T"] = proj_T(w[1], d["m2T"], f"k2T{par}")
                d["q2T"] = d["m2T"]
                d["q1T"] = d["m1T"]
            else:
                _, b = sg
                w = d["w"]
                # one bulk transpose of av from DRAM, then cast to fp8
                nc.sync.dma_start_transpose(out=avT[b][:, :, :], in_=avscr[b])
                nc.vector.tensor_copy(avT8[b][:, :, :], avT[b][:, :, :])
                d["k1T"] = proj_T(w[0], d["xT"], f"k1T{par}")
                d["q2T"] = avT8[b]

        def emit_proj_v(si):
            sg = stages[si]
            par = si % 2
            d = st[si]
            if sg[0] == "sym":
                w = d["w"]
                d["v1"] = proj_N(w[2], d["m1T"], f"v1{par}")
                d["v2"] = proj_N(w[3], d["m2T"], f"v2{par}")
            else:
                w = d["w"]
                d["v1"] = proj_N(w[1], d["xT"], f"v1{par}")

        def emit_scores(si):
            sg = stages[si]
            d = st[si]
            if sg[0] == "sym":
                d["p1"] = score_phase(d["q2T"], d["k1T"], ebias[:, 0:1])
                d["p2"] = score_phase(d["q1T"], d["k2T"], ebias[:, 0:1])
            else:
                d["p1"] = score_phase(d["q2T"], d["k1T"], cbias[:, 0:1])

        def emit_sums(si):
            d = st[si]
            ps = [d["p1"]] + ([d["p2"]] if "p2" in d else [])
            d["rc_nat"] = sums_phase(ps)

        def emit_pv(si):
            sg = stages[si]
            d = st[si]
            if sg[0] == "sym":
                _, blk, b, n1, n2, col = sg
                o1r = projp.tile([P, LT, A], F32, tag="o1r", name="o1r")

                def writer1(qt, po, rc, blk=blk, b=b):
                    res_t = smallp.tile([P, A], F32, tag="res_t", name="res_t")
                    nc.sync.dma_start(
                        out=res_t[:, :],
                        in_=res[blk, b, qt * P:(qt + 1) * P, :])
                    nc.vector.scalar_tensor_tensor(
                        o1r[:, qt, :], po, rc, res_t[:, :], op0=MULT, op1=ADD)

                def writer2(qt, po, rc, blk=blk, b=b, col=col):
                    out_t = smallp.tile([P, A], F32, tag="out_t", name="out_t")
                    nc.vector.scalar_tensor_tensor(
                        out_t[:, :], po, rc, o1r[:, qt, :], op0=MULT, op1=ADD)
                    nc.sync.dma_start(
                        out=out[b, qt * P:(qt + 1) * P, col * A:(col + 1) * A],
                        in_=out_t[:, :])
                    if blk == 1:
                        av_bf = smallp.tile([P, A], BF16, tag="av_bf",
                                            name="av_bf", bufs=8)
                        nc.vector.tensor_copy(av_bf[:, :], out_t[:, :])
                        nc.sync.dma_start(
                            out=avscr[b, qt * P:(qt + 1) * P, :],
                            in_=av_bf[:, :])

                pv_phase([(d["p1"], d["v1"], writer1),
                          (d["p2"], d["v2"], writer2)], d["rc_nat"])
            else:
                _, b = sg

                def writer_c(qt, po, rc, b=b):
                    out_t = smallp.tile([P, A], F32, tag="out_t", name="out_tc")
                    nc.scalar.activation(out_t[:, :], po, COPY, scale=rc)
                    nc.sync.dma_start(
                        out=out[b, qt * P:(qt + 1) * P, 3 * A:4 * A],
                        in_=out_t[:, :])

                pv_phase([(d["p1"], d["v1"], writer_c)], d["rc_nat"])

        # software pipeline: stage s+1's projections are emitted between
        # stage s's scores and PV; the sums matmuls slot between the k- and
        # v-projections so they never wait on the trailing exps
        emit_loads(0)
        emit_loads(1)
        emit_proj_k(0)
        emit_proj_v(0)
        for si in range(NS):
            emit_scores(si)
            if si + 2 < NS:
                emit_loads(si + 2)
            if si + 1 < NS:
                emit_proj_k(si + 1)
            emit_sums(si)
            if si + 1 < NS:
                emit_proj_v(si + 1)
            emit_pv(si)


_nc_cache = None
last_results = None


def _get_nc():
    global _nc_cache
    if _nc_cache is None:
        _nc_cache = _build()
    return _nc_cache


def kernel(**inputs):
    global last_results
    txt = np.asarray(inputs["txt"], dtype=np.float32)
    au = np.asarray(inputs["au"], dtype=np.float32)
    vi = np.asarray(inputs["vi"], dtype=np.float32)

    nat = {"txt": txt, "au": au, "vi": vi}
    mt8 = {n: np.ascontiguousarray(v.transpose(0, 2, 1)).astype(ml_dtypes.float8_e4m3)
           for n, v in nat.items()}
    W = {n: np.asarray(inputs[n], dtype=np.float32) for n in W_NAMES}
    wlist = []
    for blk in ("ta", "va", "tv"):
        wlist.append(GS * (W[f"{blk}_kx"].T @ W[f"{blk}_qy"]))  # G1
        wlist.append(GS * (W[f"{blk}_ky"].T @ W[f"{blk}_qx"]))  # G2
        wlist.append(W[f"{blk}_vx"].T)
        wlist.append(W[f"{blk}_vy"].T)
    wlist.append(GS * (W["tav_k"].T @ W["tav_q"]))  # Gc
    wlist.append(W["tav_v"].T)
    wt8_all = np.ascontiguousarray(np.stack(wlist)).astype(ml_dtypes.float8_e4m3)
    res_all = np.stack([txt + au, vi + au, txt + vi])

    in_maps = []
    for c in range(NCORES):
        sl = slice(c * BLOC, (c + 1) * BLOC)
        in_maps.append({
            "mt_txt": mt8["txt"][sl],
            "mt_au": mt8["au"][sl],
            "mt_vi": mt8["vi"][sl],
            "res": np.ascontiguousarray(res_all[:, sl]),
            "wt8": wt8_all,
            "ident128": np.eye(P, dtype=np.float32),
        })

    nc = _get_nc()
    last_results = run_bass_kernel_spmd(nc, in_maps, core_ids=list(range(NCORES)))
    core_out = np.concatenate(
        [np.asarray(last_results.results[c]["out"]) for c in range(NCORES)], axis=0)
    return np.concatenate([txt, au, vi, core_out], axis=-1).astype(np.float32)
